# revision 7
# baseline (speedup 1.0000x reference)
"""BNAF layer kernel for 8x Trainium2 NeuronCores (Bass/Tile).

Math (per sample s = (b, w)):
    h_w = tanh(w_w1 @ e)                         [256]
    w1  = (w_w2 @ h_w) -> [I=64, O=64]
    h_b = tanh(b_w1 @ e)
    b1  = b_w2 @ h_b                             [64]
    out[o]  = sum_i input[i] * exp(w1[i,o]) + b1[o]
    lj[o]   = logsumexp_i(w1[i,o] + logj[i])

Reformulation (o-major f' = o*64+i):
    W1a[s, f'] = w1 + logj  computed as one augmented GEMM:
      fp8 DoubleRow chunk: Ht (256 tanh outs, fp8) x (16*w2, fp8)
      bf16 chunk:          c3 = [16*logjT; 16] x [Sel(i); w_b2]
    psum = 16*W1a; P2 = exp(psum/16); lj = log(sum_i P2 per o);
    out  = sum_i g*P2 + b1,  g = input*exp(-logj_bf16) (exact cancel).

Engines: PE (GEMMs, fp8 DoubleRow), ACT (tanh/exp/b1-copy/Ln),
DVE (g-mult + tree levels 1-2), GPSIMD (tree levels 3-6 + final add).

Sharding: data-parallel over B across the 8 cores (32 b-rows each),
weights replicated. No collectives.
"""

import os
import sys

import numpy as np

# ---- problem constants (hardcoded; kernel.py must be self-contained) ----
B, W, IDIM, ODIM, WIN = 256, 64, 64, 64, 128
H2 = 2 * WIN            # 256 hidden
F = IDIM * ODIM         # 4096
NCORES = 8
BS = B // NCORES        # 32 b-rows per core
NS = BS * W             # 2048 samples per core
ST = 128                # samples per tile (partition dim)
NT = NS // ST           # 16 tiles
KC3 = IDIM + 1          # 65 rows in the bf16 logj/bias chunk

_PROG = None  # cached compiled program


def _ensure_path():
    for p in ("/opt/trn_rl_repo",):
        if p not in sys.path:
            sys.path.insert(0, p)


def _build_program(use_biases=True):
    """Build + schedule + compile the (SPMD, per-core) Bass program."""
    _ensure_path()
    import concourse.bass as bass
    import concourse.tile as tile
    from concourse import bacc, mybir

    f32 = mybir.dt.float32
    bf16 = mybir.dt.bfloat16
    fp8 = mybir.dt.float8e4
    AF = mybir.ActivationFunctionType
    ALU = mybir.AluOpType
    DR = mybir.MatmulPerfMode.DoubleRow

    nc = bacc.Bacc("TRN2", target_bir_lowering=False, debug=False,
                   num_devices=NCORES)

    repeat = int(os.environ.get("BNAF_REPEAT", "1"))
    # which tree levels (3..6) + the final add run on gpsimd. HW A/B: gpsimd
    # runs ~2x slower than the cost model, so only the small levels go there
    # (456 beat 3456 by 6% in-process; 23456 and all-DVE are far worse).
    gp_levels = set(int(c) for c in os.environ.get("BNAF_GP", "456"))
    stt_on_gp = "s" not in os.environ.get("BNAF_NOGP", "")
    # materialized g (dense step-1 mult operand) vs 0-stride broadcast AP;
    # HW A/B showed parity, so default to the broadcast (less DMA traffic)
    gx_mode = os.environ.get("BNAF_GX", "0") == "1"
    XW = 320 + (F if gx_mode else 0)
    # X-tile DMA trigger queue: "sp" keeps the ~500ns/launch HWDGE trigger
    # off the ACT engine (which is ~77% busy with exp/tanh); "act" reverts.
    xq = os.environ.get("BNAF_XQ", "sp")
    # lj store trigger queue at the tail
    ljq = os.environ.get("BNAF_LJQ", "act")

    # -------- DRAM tensors (per-core inputs) --------
    # xin packed per tile: [:, :, 0:128] = embT (rows=e, cols=s),
    # [:, :, 128:192] = g rows (rows=s), [:, 0:65, 192:320] = [16*logjT; 16].
    d_xin = nc.dram_tensor("xin", [NT, 128, XW], bf16,
                           kind="ExternalInput")
    d_w2dr = nc.dram_tensor("w2dr", [128, 2, F], fp8, kind="ExternalInput")
    d_w2c3 = nc.dram_tensor("w2c3", [KC3, F], bf16, kind="ExternalInput")
    d_bndr = nc.dram_tensor("bndr", [128, 2, ODIM], fp8, kind="ExternalInput")
    d_w1T = nc.dram_tensor("w1T", [WIN, H2], bf16, kind="ExternalInput")
    d_b1T = nc.dram_tensor("b1T", [WIN, H2], bf16, kind="ExternalInput")
    d_wb1 = nc.dram_tensor("wb1", [H2, 1], f32, kind="ExternalInput")
    d_bb1 = nc.dram_tensor("bb1", [H2, 1], f32, kind="ExternalInput")
    d_out = nc.dram_tensor("out", [NS, ODIM], f32, kind="ExternalOutput")
    d_lj = nc.dram_tensor("lj", [NS, ODIM], f32, kind="ExternalOutput")

    with tile.TileContext(nc) as tc:
        from contextlib import ExitStack
        with ExitStack() as ctx:
            singles = ctx.enter_context(tc.tile_pool(name="singles", bufs=1))
            work = ctx.enter_context(tc.tile_pool(name="work", bufs=3))
            psg2 = ctx.enter_context(
                tc.tile_pool(name="psg2", bufs=2, space="PSUM"))

            # ---- tile-0 input + static weights into SBUF ----
            # X(t0) and the GEMM1 weights go on the ACT queue ahead of the
            # bulk weight stream (SP queue) so tile 0 starts immediately.
            xeng = nc.sync if xq == "sp" else nc.scalar
            X0 = work.tile([128, XW], bf16, tag="X", name="X_pre0", bufs=3)
            xeng.dma_start(out=X0[:, 0:320], in_=d_xin[0][:, 0:320])
            if gx_mode:
                xeng.dma_start(out=X0[:, 320:XW], in_=d_xin[0][:, 320:XW])
            w1T = singles.tile([WIN, H2], bf16, tag="w1T")
            b1T = singles.tile([WIN, H2], bf16, tag="b1T")
            nc.sync.dma_start(out=w1T, in_=d_w1T[:, :])
            nc.sync.dma_start(out=b1T, in_=d_b1T[:, :])
            if use_biases:
                wb1 = singles.tile([128, 2], f32, tag="wb1")
                bb1 = singles.tile([128, 2], f32, tag="bb1")
                nc.sync.dma_start(out=wb1[:, 0:1], in_=d_wb1[0:128, :])
                nc.sync.dma_start(out=wb1[:, 1:2], in_=d_wb1[128:256, :])
                nc.sync.dma_start(out=bb1[:, 0:1], in_=d_bb1[0:128, :])
                nc.sync.dma_start(out=bb1[:, 1:2], in_=d_bb1[128:256, :])
            bndr = singles.tile([128, 2, ODIM], fp8, tag="bndr")
            # Weights stream in interleaved chunks ordered by first use so
            # tile-0's GEMM2 group A (blocks 0-3) unblocks before the group-B
            # bytes arrive: dr/c3 block 0, then blocks 1-3, then 4-7. The
            # sim's shared-DMA-device serialization otherwise delays the
            # first exp (and all DVE work) by ~5us.
            w2dr0 = singles.tile([128, 2, 512], fp8, tag="w2dr0")
            w2c30 = singles.tile([KC3, 512], bf16, tag="w2c30")
            w2drA = singles.tile([128, 2, 1536], fp8, tag="w2drA")
            w2c3A = singles.tile([KC3, 1536], bf16, tag="w2c3A")
            w2drB = singles.tile([128, 2, 2048], fp8, tag="w2drB")
            w2c3B = singles.tile([KC3, 2048], bf16, tag="w2c3B")
            nc.sync.dma_start(out=w2dr0, in_=d_w2dr[:, :, 0:512])
            nc.sync.dma_start(out=w2c30, in_=d_w2c3[:, 0:512])
            nc.sync.dma_start(out=w2drA, in_=d_w2dr[:, :, 512:2048])
            nc.sync.dma_start(out=w2c3A, in_=d_w2c3[:, 512:2048])
            nc.sync.dma_start(out=bndr, in_=d_bndr[:, :, :])
            nc.sync.dma_start(out=w2drB, in_=d_w2dr[:, :, 2048:F])
            nc.sync.dma_start(out=w2c3B, in_=d_w2c3[:, 2048:F])

            def w2dr_blk(fc):
                if fc == 0:
                    return w2dr0[:, :, :]
                if fc < 4:
                    return w2drA[:, :, (fc - 1) * 512:fc * 512]
                return w2drB[:, :, (fc - 4) * 512:(fc - 3) * 512]

            def w2c3_blk(fc):
                if fc == 0:
                    return w2c30[:, :]
                if fc < 4:
                    return w2c3A[:, (fc - 1) * 512:fc * 512]
                return w2c3B[:, (fc - 4) * 512:(fc - 3) * 512]

            accAB = singles.tile([128, 16 * 128], f32, tag="accAB")
            out_g = [singles.tile([128, 4, ODIM], f32, tag=f"outg{gi}",
                                  name=f"out_g{gi}") for gi in range(4)]

            # ======== per-tile pipeline ========
            for ti in range(repeat * NT):
                t = ti % NT

                if ti == 0:
                    X = X0
                else:
                    X = work.tile([128, XW], bf16, tag="X", name=f"X_{ti}",
                                  bufs=3)
                    # X loads go on the scalar queue, not serialized behind
                    # the big weight streams on the sync queue. In gx mode
                    # the (big) expanded-g block is a separate transfer so
                    # GEMM1 only waits for the small head of the tile input.
                    xeng.dma_start(out=X[:, 0:320], in_=d_xin[t][:, 0:320])
                    if gx_mode:
                        xeng.dma_start(out=X[:, 320:XW],
                                       in_=d_xin[t][:, 320:XW])
                et = X[:, 0:128]
                gt = X[:, 128:192]
                c3 = X[0:KC3, 192:320]

                psA = psg2.tile([128, 2048], f32, tag="g2", name=f"psA_{ti}")
                psB = psg2.tile([128, 2048], f32, tag="g2", name=f"psB_{ti}")

                # GEMM1 (both hypernets) -> psA[:, 0:512], [h, s] blocks
                for j, (lhs, hs) in enumerate((
                        (w1T, slice(0, 128)), (w1T, slice(128, 256)),
                        (b1T, slice(0, 128)), (b1T, slice(128, 256)))):
                    nc.tensor.matmul(
                        psA[:, j * 128:(j + 1) * 128],
                        lhs[:, hs], et, start=True, stop=True)

                # tanh -> Ht4 fp8 [128, 4, 128]: planes = (w0, w1, b0, b1)
                Ht4 = work.tile([128, 4, 128], fp8, tag="Ht", name=f"Ht_{ti}",
                                bufs=3)
                hin = psA[:, 0:512].rearrange("p (j s) -> p j s", s=128)
                if use_biases:
                    for j, (bias, col) in enumerate((
                            (wb1, 0), (wb1, 1), (bb1, 0), (bb1, 1))):
                        nc.scalar.activation(
                            Ht4[:, j, :], hin[:, j, :],
                            AF.Tanh, bias=bias[:, col:col + 1])
                else:
                    nc.scalar.activation(Ht4[:, :, :], hin, AF.Tanh)

                # b-net head: DoubleRow MM into psB[:, 1984:2048] (transient)
                nc.tensor.matmul(psB[:, 1984:2048], Ht4[:, 2:4, :], bndr,
                                 start=True, stop=True, perf_mode=DR)
                # b1 copy psum->sbuf with the 1/16 unscale folded in
                b1s = work.tile([128, ODIM], f32, tag="b1", name=f"b1_{ti}",
                                bufs=4)
                nc.scalar.mul(b1s, psB[:, 1984:2048], 1.0 / 16.0)

                # GEMM2: per group, fp8-DR chunk then bf16 c3 chunk
                for grp, ps in ((0, psA), (1, psB)):
                    for fi in range(4):
                        fc = grp * 4 + fi
                        nc.tensor.matmul(
                            ps[:, fi * 512:(fi + 1) * 512],
                            Ht4[:, 0:2, :], w2dr_blk(fc),
                            start=True, stop=False, perf_mode=DR)
                    for fi in range(4):
                        fc = grp * 4 + fi
                        nc.tensor.matmul(
                            ps[:, fi * 512:(fi + 1) * 512],
                            c3, w2c3_blk(fc),
                            start=False, stop=True)

                # exp (unscale by 1/16); MP = [M(4096) | P2(4096)]
                MP = work.tile([128, 2 * F], bf16, tag="MP", name=f"MP_{ti}",
                               bufs=4)
                P2 = MP[:, F:2 * F]
                nc.scalar.activation(P2[:, 0:2048], psA, AF.Exp,
                                     scale=1.0 / 16.0)
                nc.scalar.activation(P2[:, 2048:4096], psB, AF.Exp,
                                     scale=1.0 / 16.0)

                # weighted product M = g (bcast over o) * P2   [DVE].
                # gx_mode: g pre-expanded host-side -> dense step-1 operand
                # (a 0-stride broadcast AP may demote the op to 1x on HW)
                if gx_mode:
                    def g_op(lo, hi):
                        return X[:, 320 + lo:320 + hi]
                else:
                    def g_op(lo, hi):
                        return bass.AP(
                            tensor=gt.tensor, offset=gt.offset,
                            ap=[list(gt.ap[0]), [0, (hi - lo) // IDIM],
                                [1, IDIM]])
                # step-1 multiply + fused tree reduction over i (q = 128
                # pages: 64 M + 64 P2). The last two tiles run entirely on
                # DVE so the tail is not gated on the (slower) gpsimd queue.
                # For the first two tiles, the mult+tree is emitted per
                # 2048-wide half (A = o<32, B = o>=32) so the in-order DVE
                # queue starts right after exp(A) instead of stalling until
                # exp(B) lands (~2.5us earlier pipeline start).
                drain = ti >= repeat * NT - 1
                halves = 2 if ti <= 1 else 1
                acc_sl = accAB[:, t * 128:(t + 1) * 128]
                # splitting a level M/P2 across DVE+gpsimd measured HW-parity
                # with no split (146.5 vs 145.2us); default off
                split_lvl = int(os.environ.get("BNAF_SPLIT", "0"))
                for hh in range(halves):
                    sz = F // halves
                    p2h = P2[:, hh * sz:(hh + 1) * sz]
                    mh = MP[:, hh * sz:(hh + 1) * sz]
                    if not gx_mode:
                        p2h = p2h.rearrange("p (o i) -> p o i", i=IDIM)
                        mh = mh.rearrange("p (o i) -> p o i", i=IDIM)
                    nc.vector.tensor_tensor(
                        out=mh, in0=p2h, in1=g_op(hh * sz, (hh + 1) * sz),
                        op=ALU.mult)
                    if halves == 2:
                        # per-half tree: pages (g=M/P2, q=o-half 32, i)
                        src4 = MP[:, :].rearrange(
                            "p (g q i) -> p g q i", g=2, i=IDIM)[
                            :, :, hh * 32:(hh + 1) * 32, :]
                        i = IDIM
                        for lvl in range(1, 7):
                            eng = (nc.gpsimd
                                   if lvl in gp_levels and not drain
                                   else nc.vector)
                            if lvl < 6:
                                tl_ = work.tile(
                                    [128, 64 * (i // 2)], bf16,
                                    tag=f"tr{lvl}h{hh}",
                                    name=f"tr{lvl}h{hh}_{ti}", bufs=2)
                                vo = tl_[:, :].rearrange(
                                    "p (g q i) -> p g q i", g=2, i=i // 2)
                                eng.tensor_add(vo, src4[:, :, :, 0:i // 2],
                                               src4[:, :, :, i // 2:i])
                                src4 = tl_[:, :].rearrange(
                                    "p (g q i) -> p g q i", g=2, i=i // 2)
                                i //= 2
                            else:
                                accv = acc_sl.rearrange(
                                    "p (g q) -> p g q", g=2)[
                                    :, :, hh * 32:(hh + 1) * 32]
                                eng.tensor_add(accv, src4[:, :, :, 0],
                                               src4[:, :, :, 1])

                if halves == 1:
                    src = MP[:, :]
                    i = IDIM
                    for lvl in range(1, 7):
                        eng = (nc.gpsimd if lvl in gp_levels and not drain
                               else nc.vector)
                        v = src.rearrange("p (q i) -> p q i", i=i)
                        if lvl < 6:
                            tl_ = work.tile([128, 128 * (i // 2)], bf16,
                                            tag=f"tr{lvl}",
                                            name=f"tr{lvl}_{ti}",
                                            bufs=3 if lvl <= 2 else 2)
                            vo = tl_[:, :].rearrange(
                                "p (q i) -> p q i", i=i // 2)
                            if lvl == split_lvl and not drain:
                                # DVE sums the M pages, gpsimd the P2 pages
                                nc.vector.tensor_add(
                                    vo[:, 0:64, :], v[:, 0:64, 0:i // 2],
                                    v[:, 0:64, i // 2:i])
                                nc.gpsimd.tensor_add(
                                    vo[:, 64:128, :], v[:, 64:128, 0:i // 2],
                                    v[:, 64:128, i // 2:i])
                            else:
                                eng.tensor_add(vo, v[:, :, 0:i // 2],
                                               v[:, :, i // 2:i])
                            src = tl_[:, :]
                            i //= 2
                        else:
                            eng.tensor_add(acc_sl, v[:, :, 0:1][:, :, 0],
                                           v[:, :, 1:2][:, :, 0])

                # out = sum_i g*P2 + b1  (b1s already unscaled)
                eng = nc.gpsimd if stt_on_gp and not drain else nc.vector
                eng.tensor_add(out_g[t // 4][:, t % 4, :],
                               acc_sl[:, 0:ODIM], b1s)
                if t % 4 == 3:
                    gi = t // 4
                    dst = d_out[gi * 4 * ST:(gi + 1) * 4 * ST, :].rearrange(
                        "(blk p) c -> p blk c", p=ST)
                    nc.sync.dma_start(out=dst, in_=out_g[gi])

            # ======== batched log + store (one act-table swap; the barrier
            # keeps the scheduler from hoisting Ln mid-pipeline, which would
            # oscillate the act-table set). Chunked so the last chunk only
            # depends on the final two tiles; lj stores go on the ACT queue
            # to dodge head-of-line blocking behind the out-g3 store on SP.
            tc.no_sync_barrier()
            for lo, hi in ((0, 8), (8, 14), (14, 16)):
                n = hi - lo
                ljt = work.tile([128, n, ODIM], f32, tag=f"ljt{lo}",
                                name=f"ljt_{lo}")
                nc.scalar.activation(
                    ljt, bass.AP(tensor=accAB.tensor,
                                 offset=accAB.offset + lo * 128 + ODIM,
                                 ap=[accAB.ap[0], [128, n], [1, ODIM]]),
                    AF.Ln)
                dst = d_lj[lo * ST:hi * ST, :].rearrange(
                    "(blk p) c -> p blk c", p=ST)
                (nc.sync if ljq == "sp" else nc.scalar).dma_start(
                    out=dst, in_=ljt)

    nc.compile()
    return nc


def _prep_inputs(inputs):
    """Host-side prep: weight transforms + per-core shards."""
    import ml_dtypes
    bf = ml_dtypes.bfloat16
    f8 = ml_dtypes.float8_e4m3

    inp = np.asarray(inputs["input"], np.float32)
    emb = np.asarray(inputs["w_embeddings"], np.float32)
    logj = np.asarray(inputs["logj"], np.float32)
    w_w1 = np.asarray(inputs["w_w1"], np.float32)
    w_b1 = np.asarray(inputs["w_b1"], np.float32)
    w_w2 = np.asarray(inputs["w_w2"], np.float32)
    w_b2 = np.asarray(inputs["w_b2"], np.float32)
    b_w1 = np.asarray(inputs["b_w1"], np.float32)
    b_b1 = np.asarray(inputs["b_b1"], np.float32)
    b_w2 = np.asarray(inputs["b_w2"], np.float32)
    b_b2 = np.asarray(inputs["b_b2"], np.float32)

    # f' = o*64 + i  <->  f = i*64 + o
    fp = np.arange(F)
    i_ = fp % IDIM
    o_ = fp // IDIM
    old = i_ * ODIM + o_

    # DoubleRow w2: [ki, j, f] = 16 * w_w2.T[ki + 128*j, f'] in fp8
    w2p = w_w2.T[:, old]                       # [256, F]
    w2dr = (16.0 * w2p).reshape(2, 128, F).transpose(1, 0, 2).astype(f8)
    # bf16 chunk: [Sel(i); w_b2] (unscaled; X carries the 16x factor)
    w2c3 = np.zeros((KC3, F), np.float32)
    w2c3[0:IDIM, :] = (i_[None, :] == np.arange(IDIM)[:, None])
    w2c3[IDIM, :] = w_b2[old]
    # b-net head: [ki, j, o] = 16 * b_w2.T[ki + 128*j, o] (+ b_b2 via c3? no:
    # b_b2 folded by adding to psum is skipped; handle via b1s offset below)
    bndr = (16.0 * b_w2.T).reshape(2, 128, ODIM).transpose(1, 0, 2).astype(f8)

    shared = {
        "w2dr": w2dr,
        "w2c3": w2c3.astype(bf),
        "bndr": bndr,
        "w1T": w_w1.T.astype(bf).copy(),
        "b1T": b_w1.T.astype(bf).copy(),
        "wb1": w_b1.reshape(H2, 1).copy(),
        "bb1": b_b1.reshape(H2, 1).copy(),
    }

    gx_mode = os.environ.get("BNAF_GX", "0") == "1"
    XW = 320 + (F if gx_mode else 0)
    in_maps = []
    for c in range(NCORES):
        bsl = slice(c * BS, (c + 1) * BS)
        emb_c = emb[bsl].reshape(NS, WIN)
        logj_c = logj[bsl].reshape(NS, IDIM)
        inp_c = inp[bsl].reshape(NS, IDIM)
        logj_bf = logj_c.astype(bf)
        # g computed against the bf16-rounded logj => exact cancellation
        g_c = inp_c * np.exp(-logj_bf.astype(np.float32))
        g_bf = g_c.astype(bf)
        xin = np.zeros((NT, 128, XW), bf)
        xin[:, :, 0:WIN] = (emb_c.T.astype(bf)
                            .reshape(WIN, NT, ST).transpose(1, 0, 2))
        xin[:, :, WIN:WIN + IDIM] = g_bf.reshape(NT, ST, IDIM)
        xin[:, 0:IDIM, WIN + IDIM:WIN + IDIM + ST] = (
            (16.0 * logj_bf.astype(np.float32)).astype(bf)
            .T.reshape(IDIM, NT, ST).transpose(1, 0, 2))
        xin[:, IDIM, WIN + IDIM:WIN + IDIM + ST] = 16.0
        if gx_mode:
            # g expanded o-major: gx[s, o*64+i] = g[s, i]
            xin[:, :, 320:320 + F] = np.tile(g_bf, (1, ODIM)).reshape(
                NT, ST, F)
        in_maps.append({"xin": xin, **shared})
    return in_maps


def kernel(**inputs):
    global _PROG
    _ensure_path()
    in_maps = _prep_inputs(inputs)

    use_biases = any(
        np.any(np.asarray(inputs[k]) != 0)
        for k in ("w_b1", "b_b1"))
    b_b2 = np.asarray(inputs["b_b2"], np.float32)
    if _PROG is None or _PROG[0] != use_biases:
        _PROG = (use_biases, _build_program(use_biases=use_biases))
    nc = _PROG[1]

    if os.environ.get("BNAF_SIM"):
        # single-core CoreSim validation path (core 0 only)
        from concourse.bass_interp import CoreSim
        sim = CoreSim(nc, trace=False)
        for k, v in in_maps[0].items():
            sim.tensor(k)[:] = v
        sim.simulate()
        res0 = {"out": np.array(sim.tensor("out")),
                "lj": np.array(sim.tensor("lj"))}
        results = [res0] * NCORES
    else:
        from concourse.bass_utils import run_bass_kernel_spmd
        trace = bool(os.environ.get("BNAF_TRACE"))
        r = run_bass_kernel_spmd(nc, in_maps, core_ids=list(range(NCORES)),
                                 trace=trace)
        if trace:
            print(f"HW exec time: {r.exec_time_ns} ns "
                  f"(mean {r.mean_exec_time_ns})")
            if r.instructions_and_trace:
                print("trace path:", r.instructions_and_trace[1])
        results = r.results

    out = np.empty((B, W, ODIM), np.float32)
    lj = np.empty((B, W, ODIM), np.float32)
    for c in range(NCORES):
        bsl = slice(c * BS, (c + 1) * BS)
        out[bsl] = np.asarray(results[c]["out"], np.float32).reshape(
            BS, W, ODIM)
        lj[bsl] = np.asarray(results[c]["lj"], np.float32).reshape(
            BS, W, ODIM)
    # b_b2 is zero in the reference setup; add host-side if ever nonzero
    if np.any(b_b2 != 0):
        out += b_b2.reshape(1, 1, ODIM)
    return (out, lj)

